# revision 17
# baseline (speedup 1.0000x reference)
import sys

for _p in ("/opt/trn_rl_repo", "/root/.axon_site/_ro/trn_rl_repo"):
    if _p not in sys.path:
        sys.path.insert(0, _p)

import numpy as np
import jax
import jax.numpy as jnp
import ml_dtypes

from concourse import bass, mybir, tile
from concourse import bass_utils, bacc

F32 = mybir.dt.float32
BF16 = mybir.dt.bfloat16
I16 = mybir.dt.int16
RELU = mybir.ActivationFunctionType.Relu
OP_MAX = mybir.AluOpType.max
OP_ADD = mybir.AluOpType.add
OP_MULT = mybir.AluOpType.mult
AX_X = mybir.AxisListType.X
BF16NP = ml_dtypes.bfloat16

TRACE = False
LAST_RESULTS = None
LAST_EXEC_S = None

# logical weight matrices, stored transposed (cin, cout), blocked 128x128
WSPEC = {
    "sa1l1": (3, 64), "sa1l2": (64, 64), "sa1l3": (64, 128),
    "sa2l1a": (3, 128), "sa2l1b": (128, 128), "sa2l2": (128, 128), "sa2l3": (128, 256),
    "sa3l1a": (3, 256), "sa3l1b": (256, 256), "sa3l2": (256, 256), "sa3l3": (256, 512),
    "fp3za": (512, 256), "fp3l1a": (256, 256), "fp3l2": (256, 256),
    "fp2za": (256, 256), "fp2l1a": (128, 256), "fp2l2": (256, 128),
    "fp1z": (128, 128), "fp1l2": (128, 128), "fp1l3": (128, 128),
    "cls1": (128, 128), "cls2": (128, 2),
}


def _nblk(c):
    return (c + 127) // 128


# ----------------------------------------------------------------- host (jax cpu)

def _sqdist(a, b):
    return (jnp.sum(a * a, -1)[:, :, None] + jnp.sum(b * b, -1)[:, None, :]
            - 2.0 * jnp.einsum('bmc,bpc->bmp', a, b))


def _fps(xyz, npoint):
    Bn, Nn, _ = xyz.shape

    def step(carry, _):
        dist, far = carry
        centroid = jnp.take_along_axis(xyz, far[:, None, None], axis=1)
        d = jnp.sum((xyz - centroid) ** 2, -1)
        dist = jnp.minimum(dist, d)
        return (dist, jnp.argmax(dist, axis=1)), far

    init = (jnp.full((Bn, Nn), 1e10, jnp.float32), jnp.zeros((Bn,), jnp.int32))
    _, idx = jax.lax.scan(step, init, None, length=npoint)
    return jnp.transpose(idx)


_gather = jax.vmap(lambda p, i: p[i])


def _host_fn(xyz):
    l0 = jnp.transpose(xyz, (0, 2, 1))
    fi1 = _fps(l0, 2048)
    new1 = _gather(l0, fi1)
    idx1 = jax.lax.top_k(-_sqdist(new1, l0), 32)[1]
    gx1 = _gather(l0, idx1) - new1[:, :, None]
    fi2 = _fps(new1, 512)
    new2 = _gather(new1, fi2)
    idx2 = jax.lax.top_k(-_sqdist(new2, new1), 32)[1]
    gx2 = _gather(new1, idx2) - new2[:, :, None]
    fi3 = _fps(new2, 128)
    new3 = _gather(new2, fi3)
    idx3 = jax.lax.top_k(-_sqdist(new3, new2), 32)[1]
    gx3 = _gather(new2, idx3) - new3[:, :, None]

    def fpw(x1, x2):
        negd, idx = jax.lax.top_k(-_sqdist(x1, x2), 3)
        d = jnp.maximum(-negd, 1e-10)
        w = 1.0 / d
        w = w / jnp.sum(w, -1, keepdims=True)
        return idx, w

    i3f, w3 = fpw(new2, new3)
    i2f, w2 = fpw(new1, new2)
    i1f, w1 = fpw(l0, new1)
    return dict(gx1=gx1, gx2=gx2, gx3=gx3, idx2=idx2, idx3=idx3,
                i3f=i3f, w3=w3, i2f=i2f, w2=w2, i1f=i1f, w1=w1)


_HOST_JIT = None


def _host_indices(xyz):
    global _HOST_JIT
    cpu = jax.devices("cpu")[0]
    with jax.default_device(cpu):
        if _HOST_JIT is None:
            _HOST_JIT = jax.jit(_host_fn)
        res = _HOST_JIT(jax.device_put(np.asarray(xyz, np.float32), cpu))
        return {k: np.asarray(v) for k, v in res.items()}


def _prep_weights(params):
    def npa(x):
        return np.asarray(x, dtype=np.float32)

    out = {}

    def blocks(tag, W, g):
        Wt = (npa(W) * npa(g)[:, None]).T  # (cin, cout)
        cin, cout = Wt.shape
        assert (cin, cout) == WSPEC[tag], (tag, Wt.shape)
        for ki in range(0, cin, 128):
            for mi in range(0, cout, 128):
                out[f"{tag}_{ki // 128}_{mi // 128}"] = np.ascontiguousarray(
                    Wt[ki:ki + 128, mi:mi + 128]).astype(BF16NP)

    sa1, sa2, sa3 = params["sa1"], params["sa2"], params["sa3"]
    fp3, fp2, fp1, cls1 = params["fp3"], params["fp2"], params["fp1"], params["cls1"]

    blocks("sa1l1", npa(sa1[0][0]), sa1[0][1])
    blocks("sa1l2", npa(sa1[1][0]), sa1[1][1])
    blocks("sa1l3", npa(sa1[2][0]), sa1[2][1])

    W, g = npa(sa2[0][0]), sa2[0][1]
    blocks("sa2l1a", W[:, :3], g)
    blocks("sa2l1b", W[:, 3:], g)
    blocks("sa2l2", npa(sa2[1][0]), sa2[1][1])
    blocks("sa2l3", npa(sa2[2][0]), sa2[2][1])

    W, g = npa(sa3[0][0]), sa3[0][1]
    blocks("sa3l1a", W[:, :3], g)
    blocks("sa3l1b", W[:, 3:], g)
    blocks("sa3l2", npa(sa3[1][0]), sa3[1][1])
    blocks("sa3l3", npa(sa3[2][0]), sa3[2][1])

    W, g = npa(fp3[0][0]), fp3[0][1]
    blocks("fp3l1a", W[:, :256], g)
    blocks("fp3za", W[:, 256:], g)
    blocks("fp3l2", npa(fp3[1][0]), fp3[1][1])

    W, g = npa(fp2[0][0]), fp2[0][1]
    blocks("fp2l1a", W[:, :128], g)
    blocks("fp2za", W[:, 128:], g)
    blocks("fp2l2", npa(fp2[1][0]), fp2[1][1])

    blocks("fp1z", npa(fp1[0][0]), fp1[0][1])
    blocks("fp1l2", npa(fp1[1][0]), fp1[1][1])
    blocks("fp1l3", npa(fp1[2][0]), fp1[2][1])

    blocks("cls1", npa(cls1[0][0]), cls1[0][1])
    blocks("cls2", npa(params["cls2_w"]), np.ones((2,), np.float32))
    # partition-64 duplicates for the packed SA1 layers
    out["sa1l2d"] = np.ascontiguousarray(np.vstack([out["sa1l2_0_0"]] * 2))
    out["sa1l3d"] = np.ascontiguousarray(np.vstack([out["sa1l3_0_0"]] * 2))
    return out


def _wrap16(a):
    a = np.asarray(a, np.int16).reshape(-1).reshape(-1, 16).T  # (16, n/16)
    return np.ascontiguousarray(np.tile(a, (8, 1)))


def _smat(idx, w, nsrc):
    # dense scatter-weight matrix S[s, j] = w[j, k] where idx[j, k] == s
    ndst = idx.shape[0]
    S = np.zeros((nsrc, ndst), np.float32)
    np.add.at(S, (np.asarray(idx).ravel(),
                  np.repeat(np.arange(ndst), idx.shape[1])),
              np.asarray(w, np.float32).ravel())
    return S.astype(BF16NP)


# ----------------------------------------------------------------- device kernel

def _build_nc():
    nc = bacc.Bacc()
    din = {}

    def decl(name, shape, dt=F32):
        din[name] = nc.dram_tensor(name, list(shape), dt, kind="ExternalInput")

    decl("gx1", (3, 65536), BF16)
    decl("gx2", (3, 16384), BF16)
    decl("gx3", (3, 4096), BF16)
    decl("i2", (128, 1024), I16)
    decl("i3", (128, 256), I16)
    decl("f1i", (128, 1536), I16)
    decl("f1w", (1, 24576))
    decl("s3", (128, 512), BF16)
    decl("sa1l2d", (128, 64), BF16)
    decl("sa1l3d", (128, 128), BF16)
    for sc in range(4):
        decl(f"s2_{sc}", (128, 2048), BF16)
    for tag, (cin, cout) in WSPEC.items():
        for ki in range(_nblk(cin)):
            ksz = min(128, cin - ki * 128)
            for mi in range(_nblk(cout)):
                msz = min(128, cout - mi * 128)
                decl(f"{tag}_{ki}_{mi}", (ksz, msz), BF16)
    dout = nc.dram_tensor("out", [2, 8192], F32, kind="ExternalOutput")

    dma = nc.default_dma_engine

    with tile.TileContext(nc) as tc:
        with tc.tile_pool(name="wp", bufs=1) as wp, \
             tc.tile_pool(name="pp", bufs=6, space="PSUM") as pp, \
             tc.tile_pool(name="per", bufs=1) as per:

            # --- load weights/indices, build constants
            W = {}
            for tag, (cin, cout) in WSPEC.items():
                for ki in range(_nblk(cin)):
                    ksz = min(128, cin - ki * 128)
                    for mi in range(_nblk(cout)):
                        msz = min(128, cout - mi * 128)
                        nm = f"{tag}_{ki}_{mi}"
                        t = wp.tile([ksz, msz], BF16, tag=nm, name="wt")
                        dma.dma_start(out=t[:], in_=din[nm][:])
                        W[(tag, ki, mi)] = t

            ones1 = wp.tile([1, 128], F32, tag="ones1", name="ones1")
            nc.vector.memset(ones1[:], 1.0)
            w12d = wp.tile([128, 64], BF16, tag="w12d", name="w12d")
            dma.dma_start(out=w12d[:], in_=din["sa1l2d"][:])
            w13d = wp.tile([128, 128], BF16, tag="w13d", name="w13d")
            dma.dma_start(out=w13d[:], in_=din["sa1l3d"][:])

            def ldidx(nm, cols):
                t = wp.tile([128, cols], I16, tag=nm, name="idx")
                dma.dma_start(out=t[:], in_=din[nm][:])
                return t

            i2t = ldidx("i2", 1024)
            i3t = ldidx("i3", 256)
            f1it = ldidx("f1i", 1536)

            s3t = wp.tile([128, 512], BF16, tag="s3t", name="s3t")
            dma.dma_start(out=s3t[:], in_=din["s3"][:])
            s2t = []
            for sc in range(4):
                t = wp.tile([128, 2048], BF16, tag=f"s2t{sc}", name="s2t")
                dma.dma_start(out=t[:], in_=din[f"s2_{sc}"][:])
                s2t.append(t)

            def lin(dst_sl, contribs, eng):
                ps = pp.tile(list(dst_sl.shape), F32, tag="ps", name="ps")
                for i, (w, r) in enumerate(contribs):
                    nc.tensor.matmul(ps[:], w[:], r,
                                     start=(i == 0),
                                     stop=(i == len(contribs) - 1))
                if eng == "s":
                    nc.scalar.activation(dst_sl, ps[:], RELU)
                elif eng == "v":
                    nc.vector.tensor_scalar_max(dst_sl, ps[:], 0.0)
                elif eng == "cs":
                    nc.scalar.copy(dst_sl, ps[:])
                else:
                    nc.vector.tensor_copy(out=dst_sl, in_=ps[:])

            def linmax(dst_sl, contribs):
                # fused matmul -> 32-wide maxpool straight out of PSUM
                ng = dst_sl.shape[1]
                ps = pp.tile([128, ng, 32], F32, tag="ps", name="ps")
                for i, (w, r) in enumerate(contribs):
                    nc.tensor.matmul(ps[:], w[:], r,
                                     start=(i == 0),
                                     stop=(i == len(contribs) - 1))
                nc.vector.tensor_reduce(out=dst_sl, in_=ps[:], axis=AX_X,
                                        op=OP_MAX)

            # =================== SA1 ===================
            # pair table: feature plane 0, junk plane 1 (for d=2 bf16 gather)
            l1tab = per.tile([128, 2048, 2], BF16, tag="l1tab", name="l1tab")
            with tc.tile_pool(name="sa1", bufs=1) as sp:
                for t in range(8):  # chunks of 8192 cols (256 groups)
                    X = sp.tile([3, 8192], BF16, tag="X", name="X", bufs=2)
                    dma.dma_start(out=X[:], in_=din["gx1"][:, t * 8192:(t + 1) * 8192])
                    # layers 1-2: two 512-col groups packed per psum (64+64 rows)
                    A1 = sp.tile([128, 4096], BF16, tag="A1", name="A1", bufs=2)
                    for c in range(0, 4096, 512):
                        ps = pp.tile([128, 512], F32, tag="ps", name="ps")
                        nc.tensor.matmul(ps[0:64, :], W[("sa1l1", 0, 0)][:],
                                         X[:, 2 * c:2 * c + 512],
                                         start=True, stop=True)
                        nc.tensor.matmul(ps[64:128, :], W[("sa1l1", 0, 0)][:],
                                         X[:, 2 * c + 512:2 * c + 1024],
                                         start=True, stop=True)
                        nc.scalar.activation(A1[:, c:c + 512], ps[:], RELU)
                    A2 = sp.tile([128, 4096], BF16, tag="A2", name="A2", bufs=2)
                    for c in range(0, 4096, 512):
                        ps = pp.tile([128, 512], F32, tag="ps", name="ps")
                        nc.tensor.matmul(ps[0:64, :], w12d[0:64, :],
                                         A1[0:64, c:c + 512],
                                         start=True, stop=True)
                        nc.tensor.matmul(ps[64:128, :], w12d[64:128, :],
                                         A1[64:128, c:c + 512],
                                         start=True, stop=True)
                        nc.scalar.activation(A2[:, c:c + 512], ps[:], RELU)
                    Praw = sp.tile([128, 256], F32, tag="Praw", name="Praw", bufs=2)
                    for c in range(0, 4096, 512):
                        for h in range(2):
                            g0 = (2 * c + h * 512) // 32
                            ps = pp.tile([128, 16, 32], F32, tag="ps",
                                         name="ps")
                            nc.tensor.matmul(ps[:],
                                             w13d[h * 64:(h + 1) * 64, :],
                                             A2[h * 64:(h + 1) * 64, c:c + 512],
                                             start=True, stop=True)
                            nc.vector.tensor_reduce(out=Praw[:, g0:g0 + 16],
                                                    in_=ps[:], axis=AX_X,
                                                    op=OP_MAX)
                    nc.scalar.activation(l1tab[:, t * 256:(t + 1) * 256, 0],
                                         Praw[:], RELU)

            # =================== SA2 ===================
            l2taba = per.tile([128, 512, 2], BF16, tag="l2taba", name="l2taba")
            l2tabb = per.tile([128, 512, 2], BF16, tag="l2tabb", name="l2tabb")
            with tc.tile_pool(name="sa2", bufs=1) as sp:
                for t in range(4):  # chunks of 4096 cols (128 groups)
                    G2 = sp.tile([128, 4096, 2], BF16, tag="G2", name="G2", bufs=2)
                    nc.gpsimd.ap_gather(out_ap=G2[:], in_ap=l1tab[:],
                                        idxs_ap=i2t[:, t * 256:(t + 1) * 256],
                                        channels=128, num_elems=2048, d=2,
                                        num_idxs=4096)
                    X2 = sp.tile([3, 4096], BF16, tag="X2", name="X2", bufs=2)
                    dma.dma_start(out=X2[:], in_=din["gx2"][:, t * 4096:(t + 1) * 4096])
                    B1 = sp.tile([128, 4096], BF16, tag="B1", name="B1", bufs=2)
                    for n in range(0, 4096, 512):
                        lin(B1[:, n:n + 512],
                            [(W[("sa2l1a", 0, 0)], X2[:, n:n + 512]),
                             (W[("sa2l1b", 0, 0)], G2[:, n:n + 512, 0])], "s")
                    B2 = sp.tile([128, 4096], BF16, tag="B2", name="B2", bufs=2)
                    for n in range(0, 4096, 512):
                        lin(B2[:, n:n + 512],
                            [(W[("sa2l2", 0, 0)], B1[:, n:n + 512])], "s")
                    Prawa = sp.tile([128, 128], F32, tag="Prawa", name="Prawa", bufs=2)
                    Prawb = sp.tile([128, 128], F32, tag="Prawb", name="Prawb", bufs=2)
                    for n in range(0, 4096, 512):
                        g = n // 32
                        linmax(Prawa[:, g:g + 16],
                               [(W[("sa2l3", 0, 0)], B2[:, n:n + 512])])
                        linmax(Prawb[:, g:g + 16],
                               [(W[("sa2l3", 0, 1)], B2[:, n:n + 512])])
                    nc.scalar.activation(l2taba[:, t * 128:(t + 1) * 128, 0],
                                         Prawa[:], RELU)
                    nc.scalar.activation(l2tabb[:, t * 128:(t + 1) * 128, 0],
                                         Prawb[:], RELU)

            # =================== SA3 ===================
            l3p = [per.tile([128, 128], BF16, tag=f"l3p{i}", name="l3p")
                   for i in range(4)]
            with tc.tile_pool(name="sa3", bufs=1) as sp:
                for t in range(2):  # chunks of 2048 cols (64 groups)
                    G3a = sp.tile([128, 2048, 2], BF16, tag="G3a", name="G3a", bufs=2)
                    G3b = sp.tile([128, 2048, 2], BF16, tag="G3b", name="G3b", bufs=2)
                    idx_sl = i3t[:, t * 128:(t + 1) * 128]
                    nc.gpsimd.ap_gather(out_ap=G3a[:], in_ap=l2taba[:],
                                        idxs_ap=idx_sl, channels=128,
                                        num_elems=512, d=2, num_idxs=2048)
                    nc.gpsimd.ap_gather(out_ap=G3b[:], in_ap=l2tabb[:],
                                        idxs_ap=idx_sl, channels=128,
                                        num_elems=512, d=2, num_idxs=2048)
                    X3 = sp.tile([3, 2048], BF16, tag="X3", name="X3", bufs=2)
                    dma.dma_start(out=X3[:], in_=din["gx3"][:, t * 2048:(t + 1) * 2048])
                    C1 = [sp.tile([128, 2048], BF16, tag=f"C1{m}", name="C1")
                          for m in range(2)]
                    for m in range(2):
                        for n in range(0, 2048, 512):
                            lin(C1[m][:, n:n + 512],
                                [(W[("sa3l1a", 0, m)], X3[:, n:n + 512]),
                                 (W[("sa3l1b", 0, m)], G3a[:, n:n + 512, 0]),
                                 (W[("sa3l1b", 1, m)], G3b[:, n:n + 512, 0])],
                                "s")
                    C2 = [sp.tile([128, 2048], BF16, tag=f"C2{m}", name="C2")
                          for m in range(2)]
                    for m in range(2):
                        for n in range(0, 2048, 512):
                            lin(C2[m][:, n:n + 512],
                                [(W[("sa3l2", 0, m)], C1[0][:, n:n + 512]),
                                 (W[("sa3l2", 1, m)], C1[1][:, n:n + 512])],
                                "s" if m == 1 else "v")
                    Praw3 = [sp.tile([128, 64], F32, tag=f"Praw3{m}", name="Praw3")
                             for m in range(4)]
                    for m in range(4):
                        for n in range(0, 2048, 512):
                            g = n // 32
                            linmax(Praw3[m][:, g:g + 16],
                                   [(W[("sa3l3", 0, m)], C2[0][:, n:n + 512]),
                                    (W[("sa3l3", 1, m)], C2[1][:, n:n + 512])])
                    for m in range(4):
                        nc.scalar.activation(l3p[m][:, t * 64:(t + 1) * 64],
                                             Praw3[m][:], RELU)

            # =================== FP3 ===================
            # Z3T[s, c] = sum_cin l3[cin, s] * Wza[cin, c]  (transposed skip feats)
            # WS3 = Z3T.T @ S3 folded straight into the D1 psum accumulation
            l2fa = per.tile([128, 512], BF16, tag="l2fa", name="l2fa")
            l2fb = per.tile([128, 512], BF16, tag="l2fb", name="l2fb")
            with tc.tile_pool(name="fp3", bufs=1) as sp:
                psz = pp.tile([128, 256], F32, tag="ps", name="ps")
                for m in range(2):
                    for k in range(4):
                        nc.tensor.matmul(psz[:, m * 128:(m + 1) * 128],
                                         l3p[k][:], W[("fp3za", k, m)][:],
                                         start=(k == 0), stop=(k == 3))
                Z3T = sp.tile([128, 256], BF16, tag="Z3T", name="Z3T")
                nc.scalar.copy(Z3T[:], psz[:])
                D1 = [sp.tile([128, 512], BF16, tag=f"D1{m}", name="D1")
                      for m in range(2)]
                for m in range(2):
                    ps = pp.tile([128, 512], F32, tag="ps", name="ps")
                    nc.tensor.matmul(ps[:], W[("fp3l1a", 0, m)][:],
                                     l2taba[:, :, 0], start=True, stop=False)
                    nc.tensor.matmul(ps[:], W[("fp3l1a", 1, m)][:],
                                     l2tabb[:, :, 0], start=False, stop=False)
                    nc.tensor.matmul(ps[:], Z3T[:, m * 128:(m + 1) * 128],
                                     s3t[:], start=False, stop=True)
                    if m == 0:
                        nc.scalar.activation(D1[m][:], ps[:], RELU)
                    else:
                        nc.vector.tensor_scalar_max(D1[m][:], ps[:], 0.0)
                for m, dst in enumerate((l2fa, l2fb)):
                    lin(dst[:],
                        [(W[("fp3l2", 0, m)], D1[0][:]),
                         (W[("fp3l2", 1, m)], D1[1][:])],
                        "s" if m == 1 else "v")

            # =================== FP2 ===================
            l1f = per.tile([128, 2048], BF16, tag="l1f", name="l1f")
            with tc.tile_pool(name="fp2", bufs=1) as sp:
                Z2T = [sp.tile([128, 256], BF16, tag=f"Z2T{sc}", name="Z2T")
                       for sc in range(4)]
                for sc in range(4):
                    ps = pp.tile([128, 256], F32, tag="ps", name="ps")
                    for m in range(2):
                        nc.tensor.matmul(ps[:, m * 128:(m + 1) * 128],
                                         l2fa[:, sc * 128:(sc + 1) * 128],
                                         W[("fp2za", 0, m)][:],
                                         start=True, stop=False)
                        nc.tensor.matmul(ps[:, m * 128:(m + 1) * 128],
                                         l2fb[:, sc * 128:(sc + 1) * 128],
                                         W[("fp2za", 1, m)][:],
                                         start=False, stop=True)
                    nc.scalar.copy(Z2T[sc][:], ps[:])
                E1 = [sp.tile([128, 2048], BF16, tag=f"E1{m}", name="E1")
                      for m in range(2)]
                for m in range(2):
                    for n in range(0, 2048, 512):
                        ps = pp.tile([128, 512], F32, tag="ps", name="ps")
                        nc.tensor.matmul(ps[:], W[("fp2l1a", 0, m)][:],
                                         l1tab[:, n:n + 512, 0],
                                         start=True, stop=False)
                        for sc in range(4):
                            nc.tensor.matmul(ps[:],
                                             Z2T[sc][:, m * 128:(m + 1) * 128],
                                             s2t[sc][:, n:n + 512],
                                             start=False, stop=(sc == 3))
                        if m == 0:
                            nc.scalar.activation(E1[m][:, n:n + 512], ps[:], RELU)
                        else:
                            nc.vector.tensor_scalar_max(E1[m][:, n:n + 512],
                                                        ps[:], 0.0)
                for n in range(0, 2048, 512):
                    lin(l1f[:, n:n + 512],
                        [(W[("fp2l2", 0, 0)], E1[0][:, n:n + 512]),
                         (W[("fp2l2", 1, 0)], E1[1][:, n:n + 512])], "s")

            # =================== FP1 + classifier ===================
            with tc.tile_pool(name="fp1", bufs=1) as sp:
                Z1 = sp.tile([128, 2048], F32, tag="Z1", name="Z1")
                for n in range(0, 2048, 512):
                    lin(Z1[:, n:n + 512],
                        [(W[("fp1z", 0, 0)], l1f[:, n:n + 512])], "cs")
                F1 = sp.tile([128, 8192], BF16, tag="F1", name="F1")
                for t in range(8):  # 1024 points / 3072 idxs per chunk
                    f1wt = sp.tile([1, 3072], F32, tag="f1wt", name="f1wt",
                                   bufs=2)
                    dma.dma_start(out=f1wt[:],
                                  in_=din["f1w"][:, t * 3072:(t + 1) * 3072])
                    WB1 = sp.tile([128, 1024, 3], F32, tag="WB1", name="WB1",
                                  bufs=2)
                    for c in range(0, 3072, 384):
                        psb = pp.tile([128, 128, 3], F32, tag="psb", name="psb",
                                      bufs=2)
                        nc.tensor.matmul(psb[:], ones1[:], f1wt[:, c:c + 384],
                                         start=True, stop=True)
                        nc.scalar.copy(WB1[:, c // 3:c // 3 + 128, :], psb[:])
                    G1 = sp.tile([128, 1024, 3], F32, tag="G1", name="G1",
                                 bufs=2)
                    nc.gpsimd.ap_gather(out_ap=G1[:], in_ap=Z1[:],
                                        idxs_ap=f1it[:, t * 192:(t + 1) * 192],
                                        channels=128, num_elems=2048, d=1,
                                        num_idxs=3072)
                    nc.gpsimd.tensor_tensor(out=G1[:], in0=G1[:], in1=WB1[:],
                                            op=OP_MULT)
                    R1 = sp.tile([128, 1024], F32, tag="R1", name="R1",
                                 bufs=2)
                    nc.vector.tensor_reduce(out=R1[:], in_=G1[:], axis=AX_X,
                                            op=OP_ADD)
                    nc.scalar.activation(F1[:, t * 1024:(t + 1) * 1024],
                                         R1[:], RELU)
                for n in range(0, 8192, 512):
                    T2c = sp.tile([128, 512], BF16, tag="T2c", name="T2c",
                                  bufs=3)
                    lin(T2c[:], [(W[("fp1l2", 0, 0)], F1[:, n:n + 512])], "v")
                    l0c = sp.tile([128, 512], BF16, tag="l0c", name="l0c",
                                  bufs=3)
                    lin(l0c[:], [(W[("fp1l3", 0, 0)], T2c[:])], "s")
                    Hc = sp.tile([128, 512], BF16, tag="Hc", name="Hc",
                                 bufs=3)
                    lin(Hc[:], [(W[("cls1", 0, 0)], l0c[:])], "v")
                    OUTt = sp.tile([2, 512], F32, tag="OUTt", name="OUTt",
                                   bufs=2)
                    lin(OUTt[:], [(W[("cls2", 0, 0)], Hc[:])], None)
                    dma.dma_start(out=dout[:, n:n + 512], in_=OUTt[:])

    nc.compile()
    return nc


_NC = None


def _get_nc():
    global _NC
    if _NC is None:
        _NC = _build_nc()
    return _NC


def kernel(xyz, params):
    global LAST_RESULTS
    xyz = np.asarray(xyz, np.float32)
    hd = _host_indices(xyz)
    wmap = _prep_weights(params)

    in_maps = []
    for b in range(8):
        m = dict(wmap)
        m["gx1"] = np.ascontiguousarray(
            hd["gx1"][b].reshape(65536, 3).T).astype(BF16NP)
        m["gx2"] = np.ascontiguousarray(
            hd["gx2"][b].reshape(16384, 3).T).astype(BF16NP)
        m["gx3"] = np.ascontiguousarray(
            hd["gx3"][b].reshape(4096, 3).T).astype(BF16NP)
        m["i2"] = _wrap16(hd["idx2"][b])
        m["i3"] = _wrap16(hd["idx3"][b])
        m["f1i"] = _wrap16(hd["i1f"][b])
        m["f1w"] = np.ascontiguousarray(hd["w1"][b].reshape(1, -1).astype(np.float32))
        m["s3"] = np.ascontiguousarray(_smat(hd["i3f"][b], hd["w3"][b], 128))
        s2 = _smat(hd["i2f"][b], hd["w2"][b], 512)
        for sc in range(4):
            m[f"s2_{sc}"] = np.ascontiguousarray(s2[sc * 128:(sc + 1) * 128])
        in_maps.append(m)

    nc = _get_nc()
    global LAST_EXEC_S
    import time as _time
    _t0 = _time.time()
    res = bass_utils.run_bass_kernel_spmd(nc, in_maps, list(range(8)),
                                          trace=TRACE)
    LAST_EXEC_S = _time.time() - _t0
    LAST_RESULTS = res
    return np.stack([np.asarray(res.results[b]["out"]) for b in range(8)])
